# revision 16
# baseline (speedup 1.0000x reference)
"""CPC loss kernel for Trainium2 (8 NeuronCores, SPMD data-parallel over batch N).

Math (per batch element n, handled by core n):
  Az[t]   = W @ latent[n, t]            (K*C = 3072 outputs per position)
  scores[t, k, m] = phi[s_{t,m}] . Az[t, k]   (M=128 gathered negatives)
  num[t, k]       = latent[n, 1+t+k] . Az[t, k]
  loss = mean over (n, t<500, k) of log(sum_m exp(scores) + exp(num)) - num

Device strategy per core (fp8 edition):
  - the negative table lat8 is e4m3 with channels PAIR-PERMUTED: row byte
    2p+j holds channel p + 128j. The 16-bit-granularity transpose dma_gather
    then lands sample i's channels (p, p+128) at partition p, bytes (2i, 2i+1)
    -> the two matmul contraction halves are the even/odd byte sub-lattices.
  - gather bytes halve vs bf16 (256B/idx), cutting the 2-pass DMA cost.
  - 7 positions (896 indices) per gather (ucode caps a gather at ~1008
    indices), round-robined over 4 SWDGE queues; idx arrives in 3 chunks and
    30 gather-tile bufs keep the gather engine ahead of the consumers.
  - AzT is computed via PE (bf16) and stored e4m3 in SBUF; each (t, c_half)
    exposes a contiguous 32-col weight slab (12 real k + 20 zero pad cols).
  - Per position, 4 accumulating fp8 matmuls (2 c-halves x {positives,
    negatives}) with 4-way column tiling produce [4*32, 140] score tiles in
    PSUM; exp(x-50) + reductions as in the bf16 version.
  - Final: ln(tot*2^-32), subtract num, masked partition-sum via 1-col matmul.
Host: loss = sum(partials)/48000 + 50 + 32*ln(2).
"""

import sys, os

_ABL = ""

for _p in ("/opt/trn_rl_repo", "/root/.axon_site/_ro/trn_rl_repo"):
    if _p not in sys.path:
        sys.path.append(_p)

import numpy as np
import ml_dtypes

import concourse.bass as bass
import concourse.bacc as bacc
import concourse.mybir as mybir
from concourse.tile import TileContext, add_dep_helper
from concourse import library_config

BF16 = ml_dtypes.bfloat16
FP8 = ml_dtypes.float8_e4m3

N, T, C, K, M = 8, 512, 256, 12, 128
Tp = T - K  # 500 real positions
TPAD = 512  # padded position count (32 PSUM megatiles of 16)
PB = 15  # positive-block cols per bank: shared rhs window covers 4 positions
SHIFT = 50.0  # fixed logsumexp shift; |scores| << SHIFT + 88 so exp never overflows
DENOM = N * Tp * K  # 48000

PPG = 7  # positions per gather: 7*128 = 896 indices (ucode caps one gather ~1008;
# smaller gathers measured slower: per-gather fixed desc-gen cost dominates)
NG_FULL = Tp // PPG  # 71 full gathers
LAST_P = Tp - NG_FULL * PPG  # 3 positions in the last gather
NGATH = NG_FULL + 1  # 72
IDXC = NG_FULL * (PPG * M // 16) + LAST_P * M // 16  # idx cols: 71*56 + 24 = 4000
NQ = 4  # SWDGE queues
IDX_CHUNKS = [(0, 504), (504, 2016), (2016, 4000)]  # aligned to 56-col gathers


def build_bass():
    nc = bacc.Bacc(
        "TRN2",
        target_bir_lowering=False,
        debug=False,
        enable_asserts=False,
        num_swdge_queues=NQ,
    )
    dt = mybir.dt

    lat8 = nc.dram_tensor("lat8", [N * T, C], dt.float8e4, kind="ExternalInput").ap()
    latT = nc.dram_tensor("latT", [128, 2, T], dt.bfloat16, kind="ExternalInput").ap()
    latT8 = nc.dram_tensor("latT8", [128, 2, T], dt.float8e4, kind="ExternalInput").ap()
    wT = nc.dram_tensor("wT", [128, 2, K * C], dt.bfloat16, kind="ExternalInput").ap()
    idx = nc.dram_tensor("idx", [128, IDXC], dt.int16, kind="ExternalInput").ap()
    masks = nc.dram_tensor("masks", [128, 1 + 4 * PB], dt.float32, kind="ExternalInput").ap()
    out = nc.dram_tensor("out", [1, 1], dt.float32, kind="ExternalOutput").ap()

    with TileContext(nc) as tc:
        nc.gpsimd.load_library(library_config.mlp)
        with (
            tc.tile_pool(name="const", bufs=1) as cp,
            tc.tile_pool(name="gat", bufs=30) as gp,
            tc.tile_pool(name="scr", bufs=6) as sp,
            tc.tile_pool(name="acc", bufs=1) as ap_,
        ):
            # --- constant / weight loads (7 DMAs total: sem-recycle stalls
            # serialize HWDGE queues past ~8 outstanding DMAs) --------------
            idx_t = cp.tile([128, IDXC], dt.int16)
            c0, c1 = IDX_CHUNKS[0]
            nc.sync.dma_start(idx_t[:, c0:c1], idx[:, c0:c1])
            masks_t = cp.tile([128, 1 + 4 * PB], dt.float32)
            nc.sync.dma_start(masks_t[:], masks[:])
            latT_t = cp.tile([128, 2, T], dt.bfloat16)
            nc.sync.dma_start(latT_t[:], latT[:])
            wT_t = cp.tile([128, 2, K * C], dt.bfloat16)
            nc.sync.dma_start(wT_t[:], wT[:])
            latT8_t = cp.tile([128, 2, T], dt.float8e4)
            nc.sync.dma_start(latT8_t[:], latT8[:])
            for c0, c1 in IDX_CHUNKS[1:]:
                nc.sync.dma_start(idx_t[:, c0:c1], idx[:, c0:c1])
            pmask_t = masks_t[:, 0:1]
            maskI4_t = masks_t[:, 1 : 1 + 4 * PB].rearrange("p (s j) -> p s j", s=4)
            negshift = cp.tile([128, 1], dt.float32)
            nc.vector.memset(negshift[:], -SHIFT)

            # AzT store, tile-major: azsb[p, u*256 + h*128 + q*32 + k]
            # = Az[k, h*128+p, t=4u+q] (e4m3). Each (tile u, c-half h) owns a
            # contiguous 128-col slab of 4 position sub-slabs (12 real k + 20
            # zero pad cols so the pad output partitions produce zero scores).
            azsb = ap_.tile([128, TPAD * 64], dt.float8e4)
            azsb6 = azsb.rearrange("p (u hh q j) -> p u hh q j", hh=2, q=4, j=32)
            # only the pad cols need zeroing; the Az copies write every k<12
            # col (including t>=500 tiles), so this runs concurrently
            for h_ in range(2):
                nc.vector.memset(azsb6[:, :, h_, :, K:32], 0.0)

            tot_all = ap_.tile([128, TPAD // 4], dt.float32)
            num_all = ap_.tile([128, TPAD // 4], dt.float32)

            # --- Az phase: AzT[kc, t] = sum_c' W[kc, c'] latent[n, t, c'] ----
            with tc.tile_pool(name="az_ps", bufs=2, space="PSUM") as azps:
                for b in range(2 * K):  # kc tile: kc = b*128 + p
                    k_, h_ = b // 2, b % 2
                    pa = azps.tile([128, T], dt.float32, name="pa")
                    for hp in range(2):  # contraction half
                        nc.tensor.matmul(
                            pa[:, :],
                            lhsT=wT_t[:, hp, b * 128 : (b + 1) * 128],
                            rhs=latT_t[:, hp, :],
                            start=(hp == 0),
                            stop=(hp == 1),
                        )
                    nc.scalar.copy(out=azsb6[:, :, h_, :, k_], in_=pa[:, :])

            # --- negative gathers (t < 500 only) -----------------------------
            nidx_regs = {
                PPG * M: nc.gpsimd.to_reg(PPG * M),
                LAST_P * M: nc.gpsimd.to_reg(LAST_P * M),
            }
            ng_tiles = []
            prev_gather = None
            _ngath = NGATH
            for g in range(NGATH):
                if g >= _ngath:
                    ng_tiles.append(None)
                    continue
                npos = PPG if g < NG_FULL else LAST_P
                nidx = npos * M
                if g < NG_FULL:
                    g_t = gp.tile([128, 2, PPG * M], dt.float8e4, tag="ng", name="ng")
                else:
                    g_t = gp.tile([128, 2, nidx], dt.float8e4, tag="ng_last", name="ngl")
                ics = g * (PPG * M // 16)
                gi = nc.gpsimd.dma_gather(
                    g_t[:],
                    lat8[:],
                    idx_t[:, ics : ics + nidx // 16],
                    nidx,
                    nidx_regs[nidx],
                    C,
                    transpose=True,
                    queue_num=g % NQ,
                )
                # Pin gather scheduling order: the Tile DMASW-lane round-robin
                # must stay in lockstep with queue_num (a DMASW sem is locked
                # to one SWDGE queue).
                if prev_gather is not None:
                    add_dep_helper(gi.ins, prev_gather.ins, sync=False, reason="gather order")
                prev_gather = gi
                # pair view: [p, m, j] with j = c-half (byte parity)
                ng_tiles.append(
                    g_t.rearrange("p a b -> p (a b)").rearrange(
                        "p (m two) -> p m two", two=2
                    )
                )

            # --- score megatiles ---------------------------------------------
            _nmega = TPAD // 16
            with tc.tile_pool(name="sc_ps", bufs=2, space="PSUM") as scps:
                for mega in range(_nmega):
                    P = scps.tile([128, 4, 512], dt.float32, name="P")
                    exp_i = None
                    for s in range(4):  # bank = one 4-position score tile
                        tile_idx = mega * 4 + s
                        t0 = tile_idx * 4
                        # positive cols: one shared 15-col window for all 4
                        # positions of the tile (diag shifts by col group q)
                        pt = min(1 + t0, T - PB)  # clamp pads in-bounds
                        if "nomm" in _ABL:
                            continue
                        # batched positives: strided 128-col lhsT covers the 4
                        # positions' weight slabs -> one group over all rows
                        pos_close = None
                        for h in range(2):
                            slab4 = azsb[:, tile_idx * 256 + h * 128 : tile_idx * 256 + (h + 1) * 128]
                            pos_close = nc.tensor.matmul(
                                P[:, s, 0:PB],
                                lhsT=slab4,
                                rhs=latT8_t[:, h, pt : pt + PB],
                                start=(h == 0),
                                stop=(h == 1),
                            )
                        for q in range(4):  # column group: own 2-MM group
                            t = t0 + q
                            for h in range(2):
                                if t < Tp:
                                    g, pl = t // PPG, t % PPG
                                    nrhs = ng_tiles[g][:, M * pl : M * (pl + 1), h]
                                else:  # dummy position: zero weights, any rhs
                                    nrhs = latT8_t[:, h, 0:M]
                                slab = azsb[
                                    :,
                                    tile_idx * 256 + h * 128 + q * 32 : tile_idx * 256 + h * 128 + q * 32 + 32,
                                ]
                                mm = nc.tensor.matmul(
                                    P[32 * q : 32 * q + 32, s, PB : PB + M],
                                    lhsT=slab,
                                    rhs=nrhs,
                                    start=(h == 0),
                                    stop=(h == 1),
                                    tile_position=(0, 32 * q),
                                )
                                if h == 0:
                                    # the neg group's start clears the bank's
                                    # has_written rows: order it after the
                                    # positive group closes
                                    add_dep_helper(mm.ins, pos_close.ins, sync=False, reason="pos first")
                    if "notail" in _ABL:
                        continue
                    # tot[t,k] = sum_m exp(score-50): one exp over all 4 banks
                    # bf16 exp store: halves SBUF traffic and the following
                    # tot-reduce runs in the DVE 16-bit 2x mode
                    E4 = sp.tile([128, 4, M], dt.bfloat16, tag="exp", name="exp_o")
                    exp_i = nc.scalar.activation(
                        out=E4[:],
                        in_=P[:, :, PB : PB + M],
                        func=mybir.ActivationFunctionType.Exp,
                        bias=negshift[:],
                        scale=1.0,
                    )
                    nc.vector.tensor_reduce(
                        tot_all[:, mega * 4 : mega * 4 + 4],
                        E4[:],
                        axis=mybir.AxisListType.X,
                        op=mybir.AluOpType.add,
                    )
                    # num[t,k] -> num_all (shifted diagonal of the pos block),
                    # after the exp so every bank group is closed
                    scr4 = sp.tile([128, 4, PB], dt.float32, tag="ttr", name="ttr_o")
                    mul_i = nc.vector.tensor_mul(scr4[:], P[:, :, 0:PB], maskI4_t[:])
                    add_dep_helper(mul_i.ins, exp_i.ins, sync=True, reason="groups closed")
                    nc.vector.tensor_reduce(
                        num_all[:, mega * 4 : mega * 4 + 4],
                        scr4[:],
                        axis=mybir.AxisListType.X,
                        op=mybir.AluOpType.add,
                    )
                    # fold in the positive term: tot += exp(num - 50)
                    en_t = sp.tile([128, 4], dt.float32, tag="en", name="en_t")
                    nc.scalar.activation(
                        out=en_t[:],
                        in_=num_all[:, mega * 4 : mega * 4 + 4],
                        func=mybir.ActivationFunctionType.Exp,
                        bias=negshift[:],
                        scale=1.0,
                    )
                    nc.vector.tensor_add(
                        tot_all[:, mega * 4 : mega * 4 + 4],
                        tot_all[:, mega * 4 : mega * 4 + 4],
                        en_t[:],
                    )

            # --- final reduction --------------------------------------------
            if "nofin" in _ABL:
                dummy = ap_.tile([1, 1], dt.float32)
                nc.vector.memset(dummy[:], 0.0)
                nc.sync.dma_start(out[:], dummy[:])
            else:
                NV = Tp // 4  # 125 valid score tiles
                # ln(tot * 2^-32) keeps the ACT-ln input within its 2^64 valid
                # range for extreme scores; +32*ln2 is restored on the host.
                Lt = ap_.tile([128, NV], dt.float32)
                nc.scalar.activation(
                    out=Lt[:],
                    in_=tot_all[:, :NV],
                    func=mybir.ActivationFunctionType.Ln,
                    scale=float(2.0**-32),
                )
                Dt = ap_.tile([128, NV], dt.float32)
                rs = ap_.tile([128, 1], dt.float32)
                nc.vector.tensor_sub(Dt[:], Lt[:], num_all[:, :NV])
                nc.vector.tensor_reduce(
                    rs[:],
                    Dt[:],
                    axis=mybir.AxisListType.X,
                    op=mybir.AluOpType.add,
                )
                with tc.tile_pool(name="f_ps", bufs=1, space="PSUM") as fps:
                    psf = fps.tile([1, 1], dt.float32)
                    nc.tensor.matmul(psf[:], lhsT=rs[:], rhs=pmask_t[:])
                    outsb = ap_.tile([1, 1], dt.float32)
                    nc.scalar.copy(out=outsb[:], in_=psf[:])
                    nc.sync.dma_start(out[:], outsb[:])

    nc.compile()
    return nc


# channel pair-permutation: table row byte b holds channel (b//2) + 128*(b%2)
_PERM = (np.arange(2 * C // 2).repeat(2) % 128 + 128 * (np.arange(2 * C) % 2))[:C]
# _PERM[b] = b//2 + 128*(b%2) for b in 0..255


def prep_inputs(latent, W, samps):
    """Host-side sharding + layout marshalling. Returns per-core input maps."""
    latent = np.asarray(latent, dtype=np.float32)
    W = np.asarray(W, dtype=np.float32)
    samps = np.asarray(samps).astype(np.int64).reshape(N, Tp, M)

    lat8 = np.ascontiguousarray(
        latent.reshape(N * T, C)[:, _PERM].astype(FP8)
    )
    wT = np.ascontiguousarray(
        W.T.astype(BF16).reshape(2, 128, K * C).transpose(1, 0, 2)
    )
    pmask = ((np.arange(128) % 32) < K).astype(np.float32).reshape(128, 1)
    q_arr, k_arr = np.arange(128) // 32, np.arange(128) % 32
    maskI = (
        (np.arange(15)[None, :] == (q_arr + k_arr)[:, None]) & (k_arr < K)[:, None]
    ).astype(np.float32)
    masks = np.ascontiguousarray(
        np.concatenate([pmask, np.tile(maskI, (1, 4))], axis=1)
    )

    in_maps = []
    for n in range(N):
        latT = np.ascontiguousarray(
            latent[n].T.astype(BF16).reshape(2, 128, T).transpose(1, 0, 2)
        )
        latT8 = np.ascontiguousarray(
            latent[n].T.astype(FP8).reshape(2, 128, T).transpose(1, 0, 2)
        )
        # negative gather indices, wrapped: idx[p, g*56 + s] = flat_g[s*16 + p%16]
        flat = samps[n].reshape(Tp * M).astype(np.int16)  # position-major
        wrapped = flat.reshape(IDXC, 16).T  # [16, IDXC]
        idx = np.ascontiguousarray(np.tile(wrapped, (8, 1)))
        in_maps.append(
            {
                "lat8": lat8,
                "latT": latT,
                "latT8": latT8,
                "wT": wT,
                "idx": idx,
                "masks": masks,
            }
        )
    return in_maps


_NC_CACHE = None


def kernel(latent, W, samps):
    global _NC_CACHE
    from concourse import bass_utils

    if _NC_CACHE is None:
        _NC_CACHE = build_bass()
    nc = _NC_CACHE
    in_maps = prep_inputs(latent, W, samps)
    res = bass_utils.run_bass_kernel_spmd(nc, in_maps, core_ids=list(range(N)))
    partial = sum(float(r["out"][0, 0]) for r in res.results)
    import math

    return np.float32(partial / DENOM + SHIFT + 32.0 * math.log(2.0))


# revision 18
# speedup vs baseline: 1.1919x; 1.1919x over previous
"""CPC loss kernel for Trainium2 (8 NeuronCores, SPMD data-parallel over batch N).

Math (per batch element n, handled by core n):
  Az[t]   = W @ latent[n, t]            (K*C = 3072 outputs per position)
  scores[t, k, m] = phi[s_{t,m}] . Az[t, k]   (M=128 gathered negatives)
  num[t, k]       = latent[n, 1+t+k] . Az[t, k]
  loss = mean over (n, t<500, k) of log(sum_m exp(scores) + exp(num)) - num

Device strategy per core (fp8 edition):
  - the negative table lat8 is e4m3 with channels PAIR-PERMUTED: row byte
    2p+j holds channel p + 128j. The 16-bit-granularity transpose dma_gather
    then lands sample i's channels (p, p+128) at partition p, bytes (2i, 2i+1)
    -> the two matmul contraction halves are the even/odd byte sub-lattices.
  - gather bytes halve vs bf16 (256B/idx), cutting the 2-pass DMA cost.
  - 7 positions (896 indices) per gather (ucode caps a gather at ~1008
    indices), round-robined over 4 SWDGE queues; idx arrives in 3 chunks and
    30 gather-tile bufs keep the gather engine ahead of the consumers.
  - AzT is computed via PE (bf16) and stored e4m3 in SBUF; each (t, c_half)
    exposes a contiguous 32-col weight slab (12 real k + 20 zero pad cols).
  - Per position, 4 accumulating fp8 matmuls (2 c-halves x {positives,
    negatives}) with 4-way column tiling produce [4*32, 140] score tiles in
    PSUM; exp(x-50) + reductions as in the bf16 version.
  - Final: ln(tot*2^-32), subtract num, masked partition-sum via 1-col matmul.
Host: loss = sum(partials)/48000 + 50 + 32*ln(2).
"""

import sys, os

_ABL = ""

for _p in ("/opt/trn_rl_repo", "/root/.axon_site/_ro/trn_rl_repo"):
    if _p not in sys.path:
        sys.path.append(_p)

import numpy as np
import ml_dtypes

import concourse.bass as bass
import concourse.bacc as bacc
import concourse.mybir as mybir
from concourse.tile import TileContext, add_dep_helper
from concourse import library_config

BF16 = ml_dtypes.bfloat16
FP8 = ml_dtypes.float8_e4m3

N, T, C, K, M = 8, 512, 256, 12, 128
Tp = T - K  # 500 real positions
TPAD = 512  # padded position count (32 PSUM megatiles of 16)
PB = 15  # positive-block cols per bank: shared rhs window covers 4 positions
SHIFT = 50.0  # fixed logsumexp shift; |scores| << SHIFT + 88 so exp never overflows
DENOM = N * Tp * K  # 48000

PPG = 7  # positions per gather: 7*128 = 896 indices (ucode caps one gather ~1008;
# smaller gathers measured slower: per-gather fixed desc-gen cost dominates)
NG_FULL = Tp // PPG  # 71 full gathers
LAST_P = Tp - NG_FULL * PPG  # 3 positions in the last gather
NGATH = NG_FULL + 1  # 72
IDXC = NG_FULL * (PPG * M // 16) + LAST_P * M // 16  # idx cols: 71*56 + 24 = 4000
NQ = 4  # SWDGE queues
IDX_CHUNKS = [(0, 504), (504, 2016), (2016, 4000)]  # aligned to 56-col gathers


def build_bass():
    nc = bacc.Bacc(
        "TRN2",
        target_bir_lowering=False,
        debug=False,
        enable_asserts=False,
        num_swdge_queues=NQ,
    )
    dt = mybir.dt

    lat8 = nc.dram_tensor("lat8", [N * T, C], dt.float8e4, kind="ExternalInput").ap()
    latT = nc.dram_tensor("latT", [128, 2, T], dt.bfloat16, kind="ExternalInput").ap()
    latT8 = nc.dram_tensor("latT8", [128, 2, T], dt.float8e4, kind="ExternalInput").ap()
    wT = nc.dram_tensor("wT", [128, 2, K * C], dt.bfloat16, kind="ExternalInput").ap()
    idx = nc.dram_tensor("idx", [128, IDXC], dt.int16, kind="ExternalInput").ap()
    masks = nc.dram_tensor("masks", [128, 1 + 4 * PB], dt.float32, kind="ExternalInput").ap()
    out = nc.dram_tensor("out", [1, 1], dt.float32, kind="ExternalOutput").ap()

    with TileContext(nc) as tc:
        nc.gpsimd.load_library(library_config.mlp)
        with (
            tc.tile_pool(name="const", bufs=1) as cp,
            tc.tile_pool(name="gat", bufs=30) as gp,
            tc.tile_pool(name="scr", bufs=4) as sp,
            tc.tile_pool(name="acc", bufs=1) as ap_,
        ):
            # --- constant / weight loads (7 DMAs total: sem-recycle stalls
            # serialize HWDGE queues past ~8 outstanding DMAs) --------------
            idx_t = cp.tile([128, IDXC], dt.int16)
            c0, c1 = IDX_CHUNKS[0]
            nc.sync.dma_start(idx_t[:, c0:c1], idx[:, c0:c1])
            masks_t = cp.tile([128, 1 + 4 * PB], dt.float32)
            nc.sync.dma_start(masks_t[:], masks[:])
            latT_t = cp.tile([128, 2, T], dt.bfloat16)
            nc.sync.dma_start(latT_t[:], latT[:])
            wT_t = cp.tile([128, 2, K * C], dt.bfloat16)
            nc.sync.dma_start(wT_t[:], wT[:])
            latT8_t = cp.tile([128, 2, T], dt.float8e4)
            nc.sync.dma_start(latT8_t[:], latT8[:])
            for c0, c1 in IDX_CHUNKS[1:]:
                nc.sync.dma_start(idx_t[:, c0:c1], idx[:, c0:c1])
            pmask_t = masks_t[:, 0:1]
            maskI4_t = masks_t[:, 1 : 1 + 4 * PB].rearrange("p (s j) -> p s j", s=4)
            negshift = cp.tile([128, 1], dt.float32)
            nc.vector.memset(negshift[:], -SHIFT)

            # AzT store, tile-major: azsb[p, u*256 + h*128 + q*32 + k]
            # = Az[k, h*128+p, t=4u+q] (e4m3). Each (tile u, c-half h) owns a
            # contiguous 128-col slab of 4 position sub-slabs (12 real k + 20
            # zero pad cols so the pad output partitions produce zero scores).
            azsb = ap_.tile([128, TPAD * 64], dt.float8e4)
            azsb6 = azsb.rearrange("p (u hh q j) -> p u hh q j", hh=2, q=4, j=32)
            # only the pad cols need zeroing; the Az copies write every k<12
            # col (including t>=500 tiles), so this runs concurrently
            for h_ in range(2):
                nc.vector.memset(azsb6[:, :, h_, :, K:32], 0.0)

            tot_all = ap_.tile([128, TPAD // 4], dt.float32)
            num_all = ap_.tile([128, TPAD // 4], dt.float32)

            # --- Az phase: AzT[kc, t] = sum_c' W[kc, c'] latent[n, t, c'] ----
            with tc.tile_pool(name="az_ps", bufs=2, space="PSUM") as azps:
                for b in range(2 * K):  # kc tile: kc = b*128 + p
                    k_, h_ = b // 2, b % 2
                    pa = azps.tile([128, T], dt.float32, name="pa")
                    for hp in range(2):  # contraction half
                        nc.tensor.matmul(
                            pa[:, :],
                            lhsT=wT_t[:, hp, b * 128 : (b + 1) * 128],
                            rhs=latT_t[:, hp, :],
                            start=(hp == 0),
                            stop=(hp == 1),
                        )
                    nc.scalar.copy(out=azsb6[:, :, h_, :, k_], in_=pa[:, :])

            # --- negative gathers (t < 500 only) -----------------------------
            nidx_regs = {
                PPG * M: nc.gpsimd.to_reg(PPG * M),
                LAST_P * M: nc.gpsimd.to_reg(LAST_P * M),
            }
            ng_tiles = []
            prev_gather = None
            _ngath = NGATH
            for g in range(NGATH):
                if g >= _ngath:
                    ng_tiles.append(None)
                    continue
                npos = PPG if g < NG_FULL else LAST_P
                nidx = npos * M
                if g < NG_FULL:
                    g_t = gp.tile([128, 2, PPG * M], dt.float8e4, tag="ng", name="ng")
                else:
                    g_t = gp.tile([128, 2, nidx], dt.float8e4, tag="ng_last", name="ngl")
                ics = g * (PPG * M // 16)
                gi = nc.gpsimd.dma_gather(
                    g_t[:],
                    lat8[:],
                    idx_t[:, ics : ics + nidx // 16],
                    nidx,
                    nidx_regs[nidx],
                    C,
                    transpose=True,
                    queue_num=g % NQ,
                )
                # Pin gather scheduling order: the Tile DMASW-lane round-robin
                # must stay in lockstep with queue_num (a DMASW sem is locked
                # to one SWDGE queue).
                if prev_gather is not None:
                    add_dep_helper(gi.ins, prev_gather.ins, sync=False, reason="gather order")
                prev_gather = gi
                # pair view: [p, m, j] with j = c-half (byte parity)
                ng_tiles.append(
                    g_t.rearrange("p a b -> p (a b)").rearrange(
                        "p (m two) -> p m two", two=2
                    )
                )

            # --- score megatiles ---------------------------------------------
            _nmega = TPAD // 16
            with tc.tile_pool(name="sc_ps", bufs=2, space="PSUM") as scps:
                for mega in range(_nmega):
                    P = scps.tile([128, 4, 512], dt.float32, name="P")
                    exp_i = None
                    for s in range(4):  # bank = one 4-position score tile
                        tile_idx = mega * 4 + s
                        t0 = tile_idx * 4
                        # positive cols: one shared 15-col window for all 4
                        # positions of the tile (diag shifts by col group q)
                        pt = min(1 + t0, T - PB)  # clamp pads in-bounds
                        if "nomm" in _ABL:
                            continue
                        # batched positives: strided 128-col lhsT covers the 4
                        # positions' weight slabs -> one group over all rows
                        pos_close = None
                        for h in range(2):
                            slab4 = azsb[:, tile_idx * 256 + h * 128 : tile_idx * 256 + (h + 1) * 128]
                            pos_close = nc.tensor.matmul(
                                P[:, s, 0:PB],
                                lhsT=slab4,
                                rhs=latT8_t[:, h, pt : pt + PB],
                                start=(h == 0),
                                stop=(h == 1),
                            )
                        for q in range(4):  # column group: own 2-MM group
                            t = t0 + q
                            for h in range(2):
                                if t < Tp:
                                    g, pl = t // PPG, t % PPG
                                    nrhs = ng_tiles[g][:, M * pl : M * (pl + 1), h]
                                else:  # dummy position: zero weights, any rhs
                                    nrhs = latT8_t[:, h, 0:M]
                                slab = azsb[
                                    :,
                                    tile_idx * 256 + h * 128 + q * 32 : tile_idx * 256 + h * 128 + q * 32 + 32,
                                ]
                                mm = nc.tensor.matmul(
                                    P[32 * q : 32 * q + 32, s, PB : PB + M],
                                    lhsT=slab,
                                    rhs=nrhs,
                                    start=(h == 0),
                                    stop=(h == 1),
                                    tile_position=(0, 32 * q),
                                )
                                if h == 0:
                                    # the neg group's start clears the bank's
                                    # has_written rows: order it after the
                                    # positive group closes
                                    add_dep_helper(mm.ins, pos_close.ins, sync=False, reason="pos first")
                    if "notail" in _ABL:
                        continue
                    # tot[t,k] = sum_m exp(score-50): one exp over all 4 banks
                    E4 = sp.tile([128, 4, M], dt.float32, tag="exp", name="exp_o")
                    exp_i = nc.scalar.activation(
                        out=E4[:],
                        in_=P[:, :, PB : PB + M],
                        func=mybir.ActivationFunctionType.Exp,
                        bias=negshift[:],
                        scale=1.0,
                    )
                    nc.vector.tensor_reduce(
                        tot_all[:, mega * 4 : mega * 4 + 4],
                        E4[:],
                        axis=mybir.AxisListType.X,
                        op=mybir.AluOpType.add,
                    )
                    # num[t,k] -> num_all (shifted diagonal of the pos block),
                    # after the exp so every bank group is closed
                    scr4 = sp.tile([128, 4, PB], dt.float32, tag="ttr", name="ttr_o")
                    mul_i = nc.vector.tensor_mul(scr4[:], P[:, :, 0:PB], maskI4_t[:])
                    add_dep_helper(mul_i.ins, exp_i.ins, sync=True, reason="groups closed")
                    nc.vector.tensor_reduce(
                        num_all[:, mega * 4 : mega * 4 + 4],
                        scr4[:],
                        axis=mybir.AxisListType.X,
                        op=mybir.AluOpType.add,
                    )

            # --- final reduction --------------------------------------------
            if "nofin" in _ABL:
                dummy = ap_.tile([1, 1], dt.float32)
                nc.vector.memset(dummy[:], 0.0)
                nc.sync.dma_start(out[:], dummy[:])
            else:
                NV = Tp // 4  # 125 valid score tiles
                # fold in the positive term for all tiles at once:
                # tot += exp(num - 50). Doing this here (not per megatile)
                # keeps the steady-state consumer burst off the ACT<->DVE
                # ping-pong path that paces the gather pipeline.
                en_all = ap_.tile([128, NV], dt.float32)
                nc.scalar.activation(
                    out=en_all[:],
                    in_=num_all[:, :NV],
                    func=mybir.ActivationFunctionType.Exp,
                    bias=negshift[:],
                    scale=1.0,
                )
                nc.vector.tensor_add(
                    tot_all[:, :NV], tot_all[:, :NV], en_all[:]
                )
                # ln(tot * 2^-32) keeps the ACT-ln input within its 2^64 valid
                # range for extreme scores; +32*ln2 is restored on the host.
                Lt = ap_.tile([128, NV], dt.float32)
                nc.scalar.activation(
                    out=Lt[:],
                    in_=tot_all[:, :NV],
                    func=mybir.ActivationFunctionType.Ln,
                    scale=float(2.0**-32),
                )
                Dt = ap_.tile([128, NV], dt.float32)
                rs = ap_.tile([128, 1], dt.float32)
                nc.vector.tensor_sub(Dt[:], Lt[:], num_all[:, :NV])
                nc.vector.tensor_reduce(
                    rs[:],
                    Dt[:],
                    axis=mybir.AxisListType.X,
                    op=mybir.AluOpType.add,
                )
                with tc.tile_pool(name="f_ps", bufs=1, space="PSUM") as fps:
                    psf = fps.tile([1, 1], dt.float32)
                    nc.tensor.matmul(psf[:], lhsT=rs[:], rhs=pmask_t[:])
                    outsb = ap_.tile([1, 1], dt.float32)
                    nc.scalar.copy(out=outsb[:], in_=psf[:])
                    nc.sync.dma_start(out[:], outsb[:])

    nc.compile()
    return nc


# channel pair-permutation: table row byte b holds channel (b//2) + 128*(b%2)
_PERM = (np.arange(2 * C // 2).repeat(2) % 128 + 128 * (np.arange(2 * C) % 2))[:C]
# _PERM[b] = b//2 + 128*(b%2) for b in 0..255


def prep_inputs(latent, W, samps):
    """Host-side sharding + layout marshalling. Returns per-core input maps."""
    latent = np.asarray(latent, dtype=np.float32)
    W = np.asarray(W, dtype=np.float32)
    samps = np.asarray(samps).astype(np.int64).reshape(N, Tp, M)

    lat8 = np.ascontiguousarray(
        latent.reshape(N * T, C)[:, _PERM].astype(FP8)
    )
    wT = np.ascontiguousarray(
        W.T.astype(BF16).reshape(2, 128, K * C).transpose(1, 0, 2)
    )
    pmask = ((np.arange(128) % 32) < K).astype(np.float32).reshape(128, 1)
    q_arr, k_arr = np.arange(128) // 32, np.arange(128) % 32
    maskI = (
        (np.arange(15)[None, :] == (q_arr + k_arr)[:, None]) & (k_arr < K)[:, None]
    ).astype(np.float32)
    masks = np.ascontiguousarray(
        np.concatenate([pmask, np.tile(maskI, (1, 4))], axis=1)
    )

    in_maps = []
    for n in range(N):
        latT = np.ascontiguousarray(
            latent[n].T.astype(BF16).reshape(2, 128, T).transpose(1, 0, 2)
        )
        latT8 = np.ascontiguousarray(
            latent[n].T.astype(FP8).reshape(2, 128, T).transpose(1, 0, 2)
        )
        # negative gather indices, wrapped: idx[p, g*56 + s] = flat_g[s*16 + p%16]
        flat = samps[n].reshape(Tp * M).astype(np.int16)  # position-major
        wrapped = flat.reshape(IDXC, 16).T  # [16, IDXC]
        idx = np.ascontiguousarray(np.tile(wrapped, (8, 1)))
        in_maps.append(
            {
                "lat8": lat8,
                "latT": latT,
                "latT8": latT8,
                "wT": wT,
                "idx": idx,
                "masks": masks,
            }
        )
    return in_maps


_NC_CACHE = None


def kernel(latent, W, samps):
    global _NC_CACHE
    from concourse import bass_utils

    if _NC_CACHE is None:
        _NC_CACHE = build_bass()
    nc = _NC_CACHE
    in_maps = prep_inputs(latent, W, samps)
    res = bass_utils.run_bass_kernel_spmd(nc, in_maps, core_ids=list(range(N)))
    partial = sum(float(r["out"][0, 0]) for r in res.results)
    import math

    return np.float32(partial / DENOM + SHIFT + 32.0 * math.log(2.0))
